# revision 1
# baseline (speedup 1.0000x reference)
"""Bass/Trainium2 kernel for nn_Loss: loss = -sum_i log(predictions[i, targets[i]]).

Strategy: data-parallel over the batch axis across 8 NeuronCores; each core
handles R = B/8 = 32768 rows and streams its full [R, 1024] f32 prediction
shard (128 MiB) through SBUF at DMA line rate (memory-bound regime).

Row mapping: tile i (of NJ=256) puts batch row p*NJ + i on partition p
(per-partition contiguous 4 KiB DMA chunks), so the per-tile target column
is the per-partition scalar tcolf[:, i] with no on-chip transpose.

Select: one fused scalar_tensor_tensor per tile —
    sel = (iota_cols == t) * preds ; accum_out = row-sum = picked value
(the mask has exactly one nonzero per row, so the f32 accumulation is exact).

log() is computed in near-f32 precision on the vector engine (the ACT
table Ln is only ~2e-3 accurate): decompose x = m' * 2^E with
m' in [sqrt2/2, sqrt2) via integer bit ops, then
ln x = E*ln2 + 2*atanh(s), s = (m'-1)/(m'+1), atanh via odd polynomial
in s^2 (|s| <= 0.172, s^11 term < 1e-9).  NB: the DVE ALU computes
add/sub/mult through fp32 even on int32 tiles, so integer bit math uses
only shifts/bitwise ops and every fp add operates on values < 2^24.

Raw bass (no Tile): this container's walrus rejects instructions with
attached multi-sem waits, so synchronization is explicit standalone
wait_ge + then_inc — including same-engine RAW hazards (deep pipelines),
tracked via a per-DVE-op ordering semaphore with minimal waits.

Partition reduction via a tiny f32 matmul against a -1 vector (which also
applies the negation); each core writes one f32 partial and the host sums
the 8 partials (the unshard step).
"""

import contextlib

import numpy as np

import concourse.bass as bass
import concourse.mybir as mybir
from concourse.bass_utils import run_bass_kernel_spmd

B = 262144
V = 1024
NCORES = 8
R = B // NCORES          # rows per core = 32768
P = 128                  # SBUF partitions
NJ = R // P              # tiles (and elements per partition) = 256
NBUF = 6                 # preds tile double-buffering depth

F32 = mybir.dt.float32
I32 = mybir.dt.int32
Alu = mybir.AluOpType

LN2 = 0.6931471805599453
SQRT2_MANT = float(0x3504F3)  # mantissa bits of sqrt(2)

_nc_cache = {}


class DveSeq:
    """Emit DVE ops with minimal same-engine RAW waits via one ordering sem.

    Every emitted op incs `sem` on completion; before an op whose inputs
    were produced by op #k, a standalone wait_ge(sem, k) is emitted unless
    already covered.  WAR/WAW on the in-order engine are safe.
    """

    def __init__(self, vector, sem):
        self.vector = vector
        self.sem = sem
        self.count = 0
        self.last_wait = 0
        self.producer = {}

    def wait_upto(self, k):
        if k > self.last_wait:
            self.vector.wait_ge(self.sem, k)
            self.last_wait = k

    def emit(self, reads, writes, fn):
        need = max((self.producer.get(id(t), 0) for t in reads), default=0)
        self.wait_upto(need)
        inst = fn()
        inst.then_inc(self.sem, 1)
        self.count += 1
        for t in writes:
            self.producer[id(t)] = self.count
        return inst


def build_nc(mode="poly", twords=2, nbuf=NBUF):
    """twords=2: int64 targets passed bitcast to little-endian int32 pairs;
    twords=1: native int32 targets."""
    key = (mode, twords, nbuf)
    if key in _nc_cache:
        return _nc_cache[key]

    nc = bass.Bass()
    preds = nc.dram_tensor("preds", [R * V], F32, kind="ExternalInput")
    tgt = nc.dram_tensor("tgt", [R * twords], I32, kind="ExternalInput")
    out = nc.dram_tensor("out", [1, 1], F32, kind="ExternalOutput")

    ctx = contextlib.ExitStack()
    with ctx:
        def sb(name, shape, dtype):
            return ctx.enter_context(nc.sbuf_tensor(name, shape, dtype))

        tpair = sb("tpair", [P, twords * NJ], I32)
        tcol = sb("tcol", [P, NJ], I32)
        tAf = sb("tAf", [P, NJ], F32)
        tcolf = sb("tcolf", [P, NJ], F32)
        rowi = sb("rowi", [P, 1], I32)
        rowf = sb("rowf", [P, 1], F32)
        ident = sb("ident", [P, P], F32)
        iota_i = sb("iota_i", [P, V], I32)
        iota_f = sb("iota_f", [P, V], F32)
        gbufs = [sb(f"gbuf{i}", [P, V], F32) for i in range(nbuf)]
        sels = [sb(f"sel{i}", [P, V], F32) for i in range(nbuf)]
        picked = sb("picked", [P, NJ], F32)
        eb0 = sb("eb0", [P, NJ], I32)
        mant = sb("mant", [P, NJ], I32)
        mb = sb("mb", [P, NJ], I32)
        cmp = sb("cmp", [P, NJ], F32)
        ebf = sb("ebf", [P, NJ], F32)
        e_all = sb("e_all", [P, NJ], F32)
        factor = sb("factor", [P, NJ], F32)
        m = sb("m", [P, NJ], F32)
        u = sb("u", [P, NJ], F32)
        v = sb("v", [P, NJ], F32)
        rcp = sb("rcp", [P, NJ], F32)
        s = sb("s", [P, NJ], F32)
        z = sb("z", [P, NJ], F32)
        pa = sb("pa", [P, NJ], F32)
        pb = sb("pb", [P, NJ], F32)
        z2 = sb("z2", [P, NJ], F32)
        t4 = sb("t4", [P, NJ], F32)
        q = sb("q", [P, NJ], F32)
        e_acc = sb("e_acc", [P, 1], F32)
        q_acc = sb("q_acc", [P, 1], F32)
        negones = sb("negones", [P, 1], F32)
        ef = sb("ef", [P, 1], F32)
        total = sb("total", [P, 1], F32)
        res = sb("res", [1, 1], F32)
        res_psum = ctx.enter_context(nc.psum_tensor("res_psum", [1, 1], F32))
        psumT = [
            ctx.enter_context(nc.psum_tensor(f"psumT{i}", [P, P], F32))
            for i in range(2)
        ]

        t_sem = ctx.enter_context(nc.semaphore("t_sem"))
        i_sem = ctx.enter_context(nc.semaphore("i_sem"))
        ld_sems = [ctx.enter_context(nc.semaphore(f"ld_sem{i}")) for i in range(nbuf)]
        v_sem = ctx.enter_context(nc.semaphore("v_sem"))
        d_sem = ctx.enter_context(nc.semaphore("d_sem"))
        mm_sem = ctx.enter_context(nc.semaphore("mm_sem"))
        pt_sem = ctx.enter_context(nc.semaphore("pt_sem"))
        res_sem = ctx.enter_context(nc.semaphore("res_sem"))
        out_sem = ctx.enter_context(nc.semaphore("out_sem"))
        block = ctx.enter_context(nc.Block())

        # preds viewed [R, V]; tile i loads rows {p*NJ + i} to partition p
        preds_rv = preds[:].rearrange("(r v) -> r v", v=V)

        # d_sem value after the stt of tile i completes (analytic: the DVE
        # stream is deterministic — K0 prologue ops, then one stt per tile).
        # prologue DVE ops: memset, iota_f, (tcol destride), tAf, rowf,
        # ident, 2 PSUM->tcolf copies
        K0 = 7 + (1 if twords == 2 else 0)
        stt_done = {i: K0 + i + 1 for i in range(NJ)}
        dve_marks = {}  # d_sem values PE must wait for

        @block.sync
        def _(sync):
            sync.dma_start(
                out=tpair[:], in_=tgt[:].rearrange("(p j) -> p j", p=P)
            ).then_inc(t_sem, 16)
            for i in range(0, NJ, 2):
                if i >= nbuf:
                    sync.wait_ge(d_sem, stt_done[i - nbuf])
                sync.dma_start(
                    out=gbufs[i % nbuf][:],
                    in_=preds_rv[i * P : (i + 1) * P, :],
                ).then_inc(ld_sems[i % nbuf], 16)
            sync.wait_ge(res_sem, 1)
            sync.dma_start(out=out[:], in_=res[:]).then_inc(out_sem, 16)
            sync.wait_ge(out_sem, 16)

        @block.scalar
        def _(scalar):
            for i in range(1, NJ, 2):
                if i >= nbuf:
                    scalar.wait_ge(d_sem, stt_done[i - nbuf])
                scalar.dma_start(
                    out=gbufs[i % nbuf][:],
                    in_=preds_rv[i * P : (i + 1) * P, :],
                ).then_inc(ld_sems[i % nbuf], 16)

        @block.gpsimd
        def _(gpsimd):
            gpsimd.iota(
                iota_i[:], pattern=[[1, V]], base=0, channel_multiplier=0
            ).then_inc(i_sem, 1)
            gpsimd.iota(
                rowi[:], pattern=[[0, 1]], base=0, channel_multiplier=1
            ).then_inc(i_sem, 1)

        @block.vector
        def _(vector):
            dve = DveSeq(vector, d_sem)
            E = dve.emit  # E(reads, writes, lambda)
            ts, tt, stt = (
                vector.tensor_scalar,
                vector.tensor_tensor,
                vector.scalar_tensor_tensor,
            )

            E([], [negones], lambda: vector.memset(negones[:], -1.0))
            vector.wait_ge(i_sem, 2)
            E([], [iota_f], lambda: vector.tensor_copy(
                out=iota_f[:], in_=iota_i[:]))
            vector.wait_ge(t_sem, 16)
            if twords == 2:
                tlow = tpair[:].rearrange("p (j tw) -> p j tw", tw=2)[:, :, 0:1]
                E([], [tcol], lambda: vector.tensor_copy(
                    out=tcol[:].rearrange("p (j one) -> p j one", one=1),
                    in_=tlow))
                E([tcol], [tAf], lambda: vector.tensor_copy(
                    out=tAf[:], in_=tcol[:]))
            else:
                E([], [tAf], lambda: vector.tensor_copy(
                    out=tAf[:], in_=tpair[:]))
            E([], [rowf], lambda: vector.tensor_copy(out=rowf[:], in_=rowi[:]))
            E([iota_f, rowf], [ident], lambda: ts(
                out=ident[:], in0=iota_f[:, :P], scalar1=rowf[:, 0:1],
                scalar2=None, op0=Alu.is_equal))
            dve_marks["tAf"] = dve.producer[id(tAf)]
            dve_marks["ident"] = dve.producer[id(ident)]
            # tcolf[p, 2h+hi] = t[p*NJ + 2h+hi] ... = transpose halves of tAf
            tcolf3 = tcolf[:].rearrange("p (j two) -> p j two", two=2)
            for hi in range(2):
                vector.wait_ge(pt_sem, hi + 1)
                E([], [tcolf], lambda hi=hi: vector.tensor_copy(
                    out=tcolf3[:, :, hi : hi + 1],
                    in_=psumT[hi][:].rearrange("p (j one) -> p j one", one=1)))

            # per-tile fused select: sel = (iota == t)*preds, picked = row sum
            assert dve.count == K0, (dve.count, K0)
            for i in range(NJ):
                vector.wait_ge(ld_sems[i % nbuf], 16 * (i // nbuf + 1))
                E([gbufs[i % nbuf], iota_f, tcolf], [sels[i % nbuf], picked],
                  lambda i=i: stt(
                      out=sels[i % nbuf][:],
                      in0=iota_f[:],
                      scalar=tcolf[:, i : i + 1],
                      in1=gbufs[i % nbuf][:],
                      op0=Alu.is_equal,
                      op1=Alu.mult,
                      accum_out=picked[:, i : i + 1]))
                stt_done[i] = dve.count

            # ---- log over picked [P, NJ] ----
            bits = picked[:].bitcast(I32)
            E([picked], [mant], lambda: ts(
                out=mant[:], in0=bits, scalar1=0x7FFFFF, scalar2=None,
                op0=Alu.bitwise_and))
            E([picked], [eb0], lambda: ts(
                out=eb0[:], in0=bits, scalar1=23, scalar2=None,
                op0=Alu.logical_shift_right))
            E([mant], [mb], lambda: ts(
                out=mb[:], in0=mant[:], scalar1=127 << 23, scalar2=None,
                op0=Alu.bitwise_or))
            E([mant], [cmp], lambda: ts(
                out=cmp[:], in0=mant[:], scalar1=SQRT2_MANT, scalar2=None,
                op0=Alu.is_ge))
            E([eb0], [ebf], lambda: vector.tensor_copy(out=ebf[:], in_=eb0[:]))
            # e_acc = sum(eb0 + cmp)      (all < 2^24: exact)
            E([ebf, cmp], [e_all], lambda: tt(
                out=e_all[:], in0=ebf[:], in1=cmp[:], op=Alu.add))
            E([e_all], [e_acc], lambda: vector.tensor_reduce(
                out=e_acc[:], in_=e_all[:], axis=mybir.AxisListType.X,
                op=Alu.add))
            # m' = (1.mant) * (1 - cmp/2) in [sqrt2/2, sqrt2)
            E([cmp], [factor], lambda: ts(
                out=factor[:], in0=cmp[:], scalar1=-0.5, scalar2=1.0,
                op0=Alu.mult, op1=Alu.add))
            E([mb, factor], [m], lambda: tt(
                out=m[:], in0=mb[:].bitcast(F32), in1=factor[:], op=Alu.mult))
            # s = (m-1)/(m+1); 2*atanh(s) via odd poly in z = s^2
            E([m], [u], lambda: ts(
                out=u[:], in0=m[:], scalar1=-1.0, scalar2=None, op0=Alu.add))
            E([m], [v], lambda: ts(
                out=v[:], in0=m[:], scalar1=1.0, scalar2=None, op0=Alu.add))
            E([v], [rcp], lambda: vector.reciprocal(out=rcp[:], in_=v[:]))
            E([u, rcp], [s], lambda: tt(
                out=s[:], in0=u[:], in1=rcp[:], op=Alu.mult))
            E([s], [z], lambda: tt(out=z[:], in0=s[:], in1=s[:], op=Alu.mult))
            c0, c1, c2, c3, c4 = 2.0, 2 / 3, 2 / 5, 2 / 7, 2 / 9
            E([z], [pa], lambda: ts(
                out=pa[:], in0=z[:], scalar1=c1, scalar2=c0,
                op0=Alu.mult, op1=Alu.add))
            E([z], [pb], lambda: ts(
                out=pb[:], in0=z[:], scalar1=c3, scalar2=c2,
                op0=Alu.mult, op1=Alu.add))
            E([z], [z2], lambda: tt(out=z2[:], in0=z[:], in1=z[:], op=Alu.mult))
            E([z2], [t4], lambda: ts(
                out=t4[:], in0=z2[:], scalar1=c4, scalar2=None, op0=Alu.mult))
            E([t4, pb], [t4], lambda: tt(
                out=t4[:], in0=t4[:], in1=pb[:], op=Alu.add))
            E([t4, z2], [t4], lambda: tt(
                out=t4[:], in0=t4[:], in1=z2[:], op=Alu.mult))
            E([t4, pa], [t4], lambda: tt(
                out=t4[:], in0=t4[:], in1=pa[:], op=Alu.add))
            # q_acc = sum(t4 * s)
            E([t4, s], [q], lambda: tt(
                out=q[:], in0=t4[:], in1=s[:], op=Alu.mult))
            E([q], [q_acc], lambda: vector.tensor_reduce(
                out=q_acc[:], in_=q[:], axis=mybir.AxisListType.X, op=Alu.add))
            # total = q_acc + (e_acc - 127*NJ)*ln2
            E([e_acc], [ef], lambda: ts(
                out=ef[:], in0=e_acc[:], scalar1=-127.0 * NJ, scalar2=LN2,
                op0=Alu.add, op1=Alu.mult))
            E([q_acc, ef], [total], lambda: tt(
                out=total[:], in0=q_acc[:], in1=ef[:], op=Alu.add))
            dve.wait_upto(dve.count)
            nc.vector.engine_nop().then_inc(v_sem, 1)

            vector.wait_ge(mm_sem, 1)
            vector.tensor_copy(out=res[:], in_=res_psum[:]).then_inc(res_sem, 1)

        @block.tensor
        def _(tensor):
            tensor.wait_ge(d_sem, max(dve_marks["tAf"], dve_marks["ident"]))
            for hi in range(2):
                nc.tensor.transpose(
                    out=psumT[hi][:],
                    in_=tAf[:, hi * P : (hi + 1) * P],
                    identity=ident[:],
                ).then_inc(pt_sem, 1)
            tensor.wait_ge(v_sem, 1)
            nc.tensor.matmul(
                out=res_psum[:], lhsT=total[:], rhs=negones[:],
                start=True, stop=True,
            ).then_inc(mm_sem, 1)

    _nc_cache[key] = nc
    return nc


def _make_in_maps(predictions, targets):
    predictions = np.ascontiguousarray(predictions, dtype=np.float32)
    targets = np.ascontiguousarray(targets)
    assert targets.dtype in (np.int64, np.int32), targets.dtype
    twords = 2 if targets.dtype == np.int64 else 1
    in_maps = []
    for c in range(NCORES):
        p_shard = predictions[c * R : (c + 1) * R].reshape(-1)
        t_shard = targets[c * R : (c + 1) * R].view(np.int32)
        in_maps.append({"preds": p_shard, "tgt": t_shard})
    return in_maps, twords


def _run(predictions, targets, trace=False, mode="poly", nbuf=NBUF, **kwargs):
    in_maps, twords = _make_in_maps(predictions, targets)
    nc = build_nc(mode=mode, twords=twords, nbuf=nbuf)
    res = run_bass_kernel_spmd(nc, in_maps, list(range(NCORES)), trace=trace, **kwargs)
    partials = [res.results[c]["out"][0, 0] for c in range(NCORES)]
    total = np.float32(np.sum(np.asarray(partials, dtype=np.float64)))
    return total, res


def kernel(predictions, targets):
    total, _ = _run(predictions, targets)
    return total



# revision 2
# speedup vs baseline: 14.6471x; 14.6471x over previous
"""Bass/Trainium2 kernel for nn_Loss: loss = -sum_i log(predictions[i, targets[i]]).

Strategy: data-parallel over the batch axis across 8 NeuronCores; each core
handles R = B/8 = 32768 rows.  Only one element per row is needed (128 KiB of
the 128 MiB shard), so instead of streaming the whole shard we gather exactly
those elements with an indirect (SWDGE) DMA:

  1. iota (gpsimd):  rowv[p, j] = (p*NJ + j) << 10          (row base, exact int)
  2. DVE:            idx = rowv | t     (t < 1024 occupies the low 10 bits;
                     bitwise ops are exact on int32 — no fp32-int pitfall)
  3. gpsimd indirect_dma_start: picked[p, j] = preds.flat[idx[p, j]]
     (split into chunks so Q7 descriptor generation of chunk k+1 overlaps the
     SDMA drain of chunk k)
  4. ACT engine: activation(Ln, accum_out) -> per-partition sum of ln(picked)
     (the Ln table's ~2e-3 per-element error is unbiased noise on a 262k-term
     sum; tolerance is 2e-2 on the total)
  5. PE: matmul against a -1s vector reduces over partitions and negates.

Each core writes one f32 partial; the host sums the 8 partials (the unshard).

Raw bass (no Tile): this container's walrus rejects instructions with attached
multi-sem waits, so synchronization is explicit standalone wait_ge + then_inc.
"""

import contextlib

import numpy as np

import concourse.bass as bass
import concourse.mybir as mybir
from concourse.bass_utils import run_bass_kernel_spmd

B = 262144
V = 1024
NCORES = 8
R = B // NCORES          # rows per core = 32768
P = 128                  # SBUF partitions
NJ = R // P              # elements per partition = 256
NSPLIT = 4               # indirect-DMA chunks (pipeline Q7 gen vs SDMA drain)

F32 = mybir.dt.float32
I32 = mybir.dt.int32
Alu = mybir.AluOpType
AF = mybir.ActivationFunctionType

_nc_cache = {}


def build_nc(twords=2, nsplit=NSPLIT):
    """twords=2: int64 targets passed bitcast to little-endian int32 pairs;
    twords=1: native int32 targets."""
    key = (twords, nsplit)
    if key in _nc_cache:
        return _nc_cache[key]
    assert NJ % nsplit == 0
    nch = NJ // nsplit

    nc = bass.Bass()
    preds = nc.dram_tensor("preds", [R, V], F32, kind="ExternalInput")
    tgt = nc.dram_tensor("tgt", [R * twords], I32, kind="ExternalInput")
    out = nc.dram_tensor("out", [1, 1], F32, kind="ExternalOutput")

    ctx = contextlib.ExitStack()
    with ctx:
        def sb(name, shape, dtype):
            return ctx.enter_context(nc.sbuf_tensor(name, shape, dtype))

        tpair = sb("tpair", [P, twords * NJ], I32)
        rowv = sb("rowv", [P, NJ], I32)
        idx = sb("idx", [P, NJ], I32)
        picked = sb("picked", [P, NJ], F32)
        lnp = sb("lnp", [P, NJ], F32)
        lnacc = sb("lnacc", [P, 1], F32)
        negones = sb("negones", [P, 1], F32)
        res = sb("res", [1, 1], F32)
        res_psum = ctx.enter_context(nc.psum_tensor("res_psum", [1, 1], F32))

        t_sem = ctx.enter_context(nc.semaphore("t_sem"))
        i_sem = ctx.enter_context(nc.semaphore("i_sem"))
        x_sem = ctx.enter_context(nc.semaphore("x_sem"))
        g_sem = ctx.enter_context(nc.semaphore("g_sem"))
        n_sem = ctx.enter_context(nc.semaphore("n_sem"))
        a_sem = ctx.enter_context(nc.semaphore("a_sem"))
        mm_sem = ctx.enter_context(nc.semaphore("mm_sem"))
        res_sem = ctx.enter_context(nc.semaphore("res_sem"))
        out_sem = ctx.enter_context(nc.semaphore("out_sem"))
        block = ctx.enter_context(nc.Block())

        @block.sync
        def _(sync):
            sync.dma_start(
                out=tpair[:], in_=tgt[:].rearrange("(p j) -> p j", p=P)
            ).then_inc(t_sem, 16)
            sync.wait_ge(res_sem, 1)
            sync.dma_start(out=out[:], in_=res[:]).then_inc(out_sem, 16)
            sync.wait_ge(out_sem, 16)

        @block.gpsimd
        def _(gpsimd):
            # rowv[p, j] = (p*NJ + j) * 1024  (int32-exact, < 2^25)
            gpsimd.iota(
                rowv[:], pattern=[[V, NJ]], base=0, channel_multiplier=NJ * V
            ).then_inc(i_sem, 1)
            gpsimd.wait_ge(x_sem, 1)
            for k in range(nsplit):
                gpsimd.indirect_dma_start(
                    out=picked[:, k * nch : (k + 1) * nch],
                    out_offset=None,
                    in_=preds[:, :],
                    in_offset=bass.IndirectOffsetOnAxis(
                        ap=idx[:, k * nch : (k + 1) * nch], axis=1
                    ),
                ).then_inc(g_sem, 16)

        @block.vector
        def _(vector):
            vector.memset(negones[:], -1.0).then_inc(n_sem, 1)
            vector.wait_ge(t_sem, 16)
            vector.wait_ge(i_sem, 1)
            if twords == 2:
                tlow = tpair[:].rearrange("p (j tw) -> p j tw", tw=2)[:, :, 0:1]
            else:
                tlow = tpair[:].rearrange("p (j one) -> p j one", one=1)
            vector.tensor_tensor(
                out=idx[:].rearrange("p (j one) -> p j one", one=1),
                in0=rowv[:].rearrange("p (j one) -> p j one", one=1),
                in1=tlow,
                op=Alu.bitwise_or,
            ).then_inc(x_sem, 1)
            vector.wait_ge(mm_sem, 1)
            vector.tensor_copy(out=res[:], in_=res_psum[:]).then_inc(res_sem, 1)

        @block.scalar
        def _(scalar):
            scalar.wait_ge(g_sem, 16 * nsplit)
            scalar.activation(
                out=lnp[:], in_=picked[:], func=AF.Ln, accum_out=lnacc[:]
            ).then_inc(a_sem, 1)

        @block.tensor
        def _(tensor):
            tensor.wait_ge(a_sem, 1)
            tensor.wait_ge(n_sem, 1)
            nc.tensor.matmul(
                out=res_psum[:], lhsT=lnacc[:], rhs=negones[:],
                start=True, stop=True,
            ).then_inc(mm_sem, 1)

    _nc_cache[key] = nc
    return nc


def _make_in_maps(predictions, targets):
    predictions = np.ascontiguousarray(predictions, dtype=np.float32)
    targets = np.ascontiguousarray(targets)
    assert targets.dtype in (np.int64, np.int32), targets.dtype
    twords = 2 if targets.dtype == np.int64 else 1
    in_maps = []
    for c in range(NCORES):
        p_shard = predictions[c * R : (c + 1) * R]
        t_shard = targets[c * R : (c + 1) * R].view(np.int32)
        in_maps.append({"preds": p_shard, "tgt": t_shard})
    return in_maps, twords


def _run(predictions, targets, trace=False, nsplit=NSPLIT, **kwargs):
    in_maps, twords = _make_in_maps(predictions, targets)
    nc = build_nc(twords=twords, nsplit=nsplit)
    res = run_bass_kernel_spmd(nc, in_maps, list(range(NCORES)), trace=trace, **kwargs)
    partials = [res.results[c]["out"][0, 0] for c in range(NCORES)]
    total = np.float32(np.sum(np.asarray(partials, dtype=np.float64)))
    return total, res


def kernel(predictions, targets):
    total, _ = _run(predictions, targets)
    return total


# revision 6
# speedup vs baseline: 16.1764x; 1.1044x over previous
"""Bass/Trainium2 kernel for nn_Loss: loss = -sum_i log(predictions[i, targets[i]]).

Strategy: data-parallel over the batch axis across 8 NeuronCores; each core
handles R = B/8 = 32768 rows.  Only one element per row is needed (128 KiB of
the 128 MiB shard), so instead of streaming the whole shard we gather exactly
those elements with an indirect (SWDGE) DMA.

Per core:
  scalar (ACT): HWDGE-load targets -> tpair [P, 2*NJ]; dummy Ln activation
     (scale=0, bias=1 -> ln(1), input-independent) to pull the ACT Ln table
     load off the critical path; then one Ln per gather chunk with accum_out.
  gpsimd: iota rowv[p,j] = (p*NJ+j)<<10 (exact int); idx = rowv | t (t < 1024
     occupies the low 10 bits; bitwise ops are exact on int32); then nsplit
     indirect_dma_start chunks so Q7 descriptor generation of chunk k+1
     overlaps the SDMA drain of chunk k.  Same-engine RAW hazards are bridged
     with a self-semaphore (deep pipelines).
  sync: DMA the [P, nsplit] accumulator out.

Each core returns [P, nsplit] partial sums of ln(picked); the host sums all
8*P*nsplit values and negates (the unshard step).  The ACT Ln table's ~4e-3
per-element error is far inside the 2e-2 tolerance on the 262k-term sum.

Raw bass (no Tile): this container's walrus rejects instructions with attached
multi-sem waits, so synchronization is explicit standalone wait_ge + then_inc.
"""

import contextlib

import numpy as np

import concourse.bass as bass
import concourse.mybir as mybir
from concourse.bass_utils import run_bass_kernel_spmd

B = 262144
V = 1024
NCORES = 8
R = B // NCORES          # rows per core = 32768
P = 128                  # SBUF partitions
NJ = R // P              # elements per partition = 256
NSPLIT = 4               # indirect-DMA chunks (pipeline Q7 gen vs SDMA drain)

F32 = mybir.dt.float32
I32 = mybir.dt.int32
Alu = mybir.AluOpType
AF = mybir.ActivationFunctionType

_nc_cache = {}


def build_nc(twords=2, nsplit=NSPLIT):
    """twords=2: int64 targets passed bitcast to little-endian int32 pairs;
    twords=1: native int32 targets."""
    key = (twords, nsplit)
    if key in _nc_cache:
        return _nc_cache[key]
    assert NJ % nsplit == 0
    nch = NJ // nsplit

    nc = bass.Bass()
    preds = nc.dram_tensor("preds", [R, V], F32, kind="ExternalInput")
    tgt = nc.dram_tensor("tgt", [R * twords], I32, kind="ExternalInput")
    out = nc.dram_tensor("out", [P, nsplit], F32, kind="ExternalOutput")

    ctx = contextlib.ExitStack()
    with ctx:
        def sb(name, shape, dtype):
            return ctx.enter_context(nc.sbuf_tensor(name, shape, dtype))

        tpair = sb("tpair", [P, twords * NJ], I32)
        rowv = sb("rowv", [P, NJ], I32)
        idx = sb("idx", [P, NJ], I32)
        picked = sb("picked", [P, NJ], F32)
        lnp = sb("lnp", [P, NJ], F32)
        lnacc = sb("lnacc", [P, nsplit], F32)
        warm = sb("warm", [P, 1], F32)

        t_sem = ctx.enter_context(nc.semaphore("t_sem"))
        i_sem = ctx.enter_context(nc.semaphore("i_sem"))
        x_sem = ctx.enter_context(nc.semaphore("x_sem"))
        g_sem = ctx.enter_context(nc.semaphore("g_sem"))
        a_sem = ctx.enter_context(nc.semaphore("a_sem"))
        out_sem = ctx.enter_context(nc.semaphore("out_sem"))
        block = ctx.enter_context(nc.Block())

        @block.scalar
        def _(scalar):
            # ln(0*x + 1) = 0: input-independent; forces the Ln table load now
            scalar.activation(
                out=warm[:], in_=warm[:], func=AF.Ln, bias=1.0, scale=0.0
            )
            for k in range(nsplit):
                scalar.wait_ge(g_sem, 16 * (k + 1))
                scalar.activation(
                    out=lnp[:, k * nch : (k + 1) * nch],
                    in_=picked[:, k * nch : (k + 1) * nch],
                    func=AF.Ln,
                    accum_out=lnacc[:, k : k + 1],
                ).then_inc(a_sem, 1)

        @block.vector
        def _(vector):
            vector.wait_ge(t_sem, 16)
            vector.wait_ge(i_sem, 1)
            if twords == 2:
                tlow = tpair[:].rearrange("p (j tw) -> p j tw", tw=2)[:, :, 0:1]
            else:
                tlow = tpair[:].rearrange("p (j one) -> p j one", one=1)
            vector.tensor_tensor(
                out=idx[:].rearrange("p (j one) -> p j one", one=1),
                in0=rowv[:].rearrange("p (j one) -> p j one", one=1),
                in1=tlow,
                op=Alu.bitwise_or,
            ).then_inc(x_sem, 1)

        @block.gpsimd
        def _(gpsimd):
            # rowv[p, j] = (p*NJ + j) * 1024  (int32-exact, < 2^25)
            gpsimd.iota(
                rowv[:], pattern=[[V, NJ]], base=0, channel_multiplier=NJ * V
            ).then_inc(i_sem, 1)
            gpsimd.wait_ge(x_sem, 1)
            for k in range(nsplit):
                gpsimd.indirect_dma_start(
                    out=picked[:, k * nch : (k + 1) * nch],
                    out_offset=None,
                    in_=preds[:, :],
                    in_offset=bass.IndirectOffsetOnAxis(
                        ap=idx[:, k * nch : (k + 1) * nch], axis=1
                    ),
                ).then_inc(g_sem, 16)

        @block.sync
        def _(sync):
            sync.dma_start(
                out=tpair[:], in_=tgt[:].rearrange("(p j) -> p j", p=P)
            ).then_inc(t_sem, 16)
            sync.wait_ge(a_sem, nsplit)
            sync.dma_start(out=out[:], in_=lnacc[:]).then_inc(out_sem, 16)
            sync.wait_ge(out_sem, 16)

    _nc_cache[key] = nc
    return nc


def _make_in_maps(predictions, targets):
    predictions = np.ascontiguousarray(predictions, dtype=np.float32)
    targets = np.ascontiguousarray(targets)
    assert targets.dtype in (np.int64, np.int32), targets.dtype
    twords = 2 if targets.dtype == np.int64 else 1
    in_maps = []
    for c in range(NCORES):
        p_shard = predictions[c * R : (c + 1) * R]
        t_shard = targets[c * R : (c + 1) * R].view(np.int32)
        in_maps.append({"preds": p_shard, "tgt": t_shard})
    return in_maps, twords


def _run(predictions, targets, trace=False, nsplit=NSPLIT, **kwargs):
    in_maps, twords = _make_in_maps(predictions, targets)
    nc = build_nc(twords=twords, nsplit=nsplit)
    res = run_bass_kernel_spmd(nc, in_maps, list(range(NCORES)), trace=trace, **kwargs)
    acc = np.zeros((), dtype=np.float64)
    for c in range(NCORES):
        acc += np.sum(res.results[c]["out"].astype(np.float64))
    return np.float32(-acc), res


def kernel(predictions, targets):
    total, _ = _run(predictions, targets)
    return total
